# revision 2
# baseline (speedup 1.0000x reference)
"""EnsembleGRU Trainium2 kernel, v2: PE-assisted scan.

Math (per ensemble member e, H=1):
    gi = x @ Wc^T + bc   (Wc = Wih @ Wl folded on host; bc likewise)
    scan over W steps:
        r  = sigmoid(gi_r + a*h)            a = whh[0]
        z  = sigmoid(gi_z + b*h)            b = whh[1]
        n  = tanh(gi_n + r*(c*h + d))       c = whh[2], d = bhh[2]
        h' = (1-z)*n + z*h = q - u,  q = z*h, u = (z-1)*n

v2 scan structure: the gate-arg constructions run on the TensorEngine as
tiny accumulating diag matmuls into the PSUM gi regions:
    gi_r[w+1] += diag(a)*q(w)  (early, after sigma)
    gi_r[w+1] += diag(-a)*u(w) (late, after tanh)   => + a*h'(w)
    gi_n[w]   += diag(c)*v(w),  v = h*r
so sigma/tanh read finished args straight from PSUM and the DVE only does
v, q, u, h' per step. Group matmuls are sliced across scan steps to keep
the in-order PE queue from blocking the latency-critical assist matmuls.

Sharding: E=16 members over 8 cores (2 per core), zero communication.
Lane layout per core: partition p = e_loc*64 + p' (p' in 0..63),
free col c in 0..39, bi = p'*40 + c  (5120 lanes = 128 x 40).
"""

import numpy as np

W, E, B, I, F = 64, 16, 256, 10, 8
BI = B * I            # 2560
NCORES = 8
E_LOC = E // NCORES   # 2
PP = 64               # partitions per member
CC = BI // PP         # 40 free cols per step
G = 3                 # gates

WG = 8                # steps per gi matmul group
NGRP = W // WG
# diag slot layout (r-gate first so its DMA slice can land before the rest):
#   [bias_r, f0-7_r, bias_z, f0-7_z, bias_n, f0-7_n, a, b, -a, -b, c]
D_A, D_B, D_NA, D_NB, D_C = 27, 28, 29, 30, 31
NDIAG = 32
NDG_HEAD = 9  # bias_r + 8 r-gate f diags: DMA'd as the first slice


def _dslot(g, f):
    """diag slot for gate g, f-term f (f == -1 -> bias)."""
    return g * (F + 1) + 1 + f

_CACHED = {}


def _build_nc(d_nonzero: bool, rep: int = 1):
    import contextlib

    import concourse.bacc as bacc
    import concourse.mybir as mybir
    from concourse.tile import TileContext

    AL = mybir.AluOpType
    AF = mybir.ActivationFunctionType
    f32 = mybir.dt.float32
    f16 = mybir.dt.float16

    nc = bacc.Bacc("TRN2", target_bir_lowering=False)

    xh = nc.dram_tensor("xh", [128, F, W, CC], f16, kind="ExternalInput")
    dg = nc.dram_tensor("dg", [128, NDIAG * 128], f16, kind="ExternalInput")
    cst = nc.dram_tensor("cst", [128, 7 + CC], f32, kind="ExternalInput")
    out = nc.dram_tensor("out", [128, W * CC], f32, kind="ExternalOutput")

    with TileContext(nc) as tc:
        with (
            tc.tile_pool(name="const", bufs=1) as constp,
            tc.tile_pool(name="xp", bufs=2) as xp,
            tc.tile_pool(name="gip", bufs=2, space="PSUM") as gip,
            tc.tile_pool(name="warmp", bufs=1, space="PSUM") as warmp,
            tc.tile_pool(name="scan", bufs=3) as scanp,
            tc.tile_pool(name="outp", bufs=1) as outp,
        ):
            dg_sb = constp.tile([128, NDIAG * 128], f16, tag="dg")
            cst_sb = constp.tile([128, 7 + CC], f32, tag="cst")
            ones = constp.tile([128, WG * CC], f16, tag="ones")
            h0h = constp.tile([128, CC], f16, tag="h0h")
            out_sb = outp.tile([128, (W + 1) * CC], f32, tag="out")

            # dg/cst on the Activation queue so the x loads (sync queue)
            # start immediately; r-gate diag slice first so group 0's
            # r mains can start as soon as x0 lands
            nc.scalar.dma_start(dg_sb[:, : NDG_HEAD * 128], dg[:, : NDG_HEAD * 128])
            nc.scalar.dma_start(dg_sb[:, NDG_HEAD * 128 :], dg[:, NDG_HEAD * 128 :])
            nc.scalar.dma_start(cst_sb[:], cst[:])
            nc.vector.memset(ones[:], 1.0)
            # h0 into slot 0 (f32) and fp16 copy for the PE assists
            nc.vector.tensor_copy(out_sb[:, 0:CC], cst_sb[:, 7 : 7 + CC])
            nc.vector.tensor_copy(h0h[:], cst_sb[:, 7 : 7 + CC])

            c_s = cst_sb[:, 2:3]
            d_s = cst_sb[:, 3:4]

            # PE p-state warm-up: contiguous dummy matmuls during the input
            # DMA window so group 0's mains run at full clock
            warm = warmp.tile([128, 512], f32, tag="warm")
            for _ in range(10):
                nc.tensor.matmul(warm[:, : 5 * CC], ones[:, :128],
                                 ones[:, : 5 * CC], start=True, stop=True,
                                 skip_group_check=True)

            loop_cm = tc.For_i(0, rep, 1) if rep > 1 else contextlib.nullcontext()
            with loop_cm:
                _body(
                    nc, tc, xp, gip, scanp, xh, out, dg_sb, ones, h0h, out_sb,
                    c_s, d_s, AL, AF, f32, f16, d_nonzero,
                )

    nc.finalize()
    return nc


def _body(
    nc, tc, xp, gip, scanp, xh, out, dg_sb, ones, h0h, out_sb,
    c_s, d_s, AL, AF, f32, f16, d_nonzero,
):
    def diag(i):
        return dg_sb[:, i * 128 : (i + 1) * 128]

    gi_tiles = {}
    x_tiles = {}

    def emit_x_dma(k):
        x_t = xp.tile([128, F * WG * CC], f16, tag="x")
        x_tiles[k] = x_t
        nc.sync.dma_start(
            x_t[:].rearrange("p (f w c) -> p f w c", f=F, c=CC),
            xh[:, :, k * WG : (k + 1) * WG, :],
        )

    def alloc_gi(k):
        gi_tiles[k] = gip.tile([128, 3 * 512], f32, tag="gi", name="gi")

    # one main-matmul slice: (g, f) with f == -1 meaning the bias matmul
    def emit_main_mm(k, g, f):
        gi_ps = gi_tiles[k]
        reg = gi_ps[:, g * 512 : g * 512 + WG * CC]
        if f < 0:
            nc.tensor.matmul(
                reg, diag(_dslot(g, -1)), ones[:, : WG * CC],
                start=True, stop=False, skip_group_check=True,
            )
        else:
            nc.tensor.matmul(
                reg, diag(_dslot(g, f)),
                x_tiles[k][:, f * WG * CC : (f + 1) * WG * CC],
                start=False, stop=(f == F - 1), skip_group_check=True,
            )

    # main-mm schedule: region-major (r fully, then z, then n) so the
    # earliest-read regions complete first when sliced across steps
    MAIN_SEQ = [(g, f) for g in range(G) for f in [-1] + list(range(F))]
    NMAIN = len(MAIN_SEQ)  # 27

    def gi_ap(w, g):
        k, wl = divmod(w, WG)
        return gi_tiles[k][:, g * 512 + wl * CC : g * 512 + (wl + 1) * CC]

    def gi_rz_ap(w):
        k, wl = divmod(w, WG)
        t = gi_tiles[k][:]
        return t.rearrange("p (g x) -> p g x", g=3)[:, 0:2, wl * CC : (wl + 1) * CC]

    def emit_out_dma(k):
        nc.sync.dma_start(
            out[:, k * WG * CC : (k + 1) * WG * CC],
            out_sb[:, (k * WG + 1) * CC : ((k + 1) * WG + 1) * CC],
        )

    # prologue: group 0 mains fully + h0 assists; group k>=1 mains are
    # sliced across group k-1's steps (1-group lookahead: the PSUM buf WAR
    # vs group k-2 is already clear, so slices run in each step's PE idle
    # instead of bursting at the boundary).
    emit_x_dma(0)
    emit_x_dma(1)
    alloc_gi(0)
    for g, f in MAIN_SEQ:
        emit_main_mm(0, g, f)
    nc.tensor.matmul(gi_ap(0, 0), diag(D_A), h0h[:], start=False, stop=True,
                     skip_group_check=True)
    nc.tensor.matmul(gi_ap(0, 1), diag(D_B), h0h[:], start=False, stop=True,
                     skip_group_check=True)

    # front-loaded slice sizes per step of the previous group
    SLICE_SIZES = [4, 4, 4, 4, 4, 3, 2, 2]
    SLICE_LO = [sum(SLICE_SIZES[:i]) for i in range(WG + 1)]

    def slice_for_step(w):
        k, wl = divmod(w, WG)
        kk = k + 1
        if kk >= NGRP:
            return []
        return [(kk, g, f) for (g, f) in MAIN_SEQ[SLICE_LO[wl]:SLICE_LO[wl + 1]]]

    for w in range(W):
        k, wl = divmod(w, WG)
        if w > 0 and wl == 0:
            emit_out_dma(k - 1)
        if wl == 0:
            if k + 1 < NGRP:
                alloc_gi(k + 1)
            if k + 2 < NGRP:
                emit_x_dma(k + 2)

        h = out_sb[:, w * CC : (w + 1) * CC]
        rz = scanp.tile([128, 2 * CC], f32, tag="rz")
        n_t = scanp.tile([128, CC], f32, tag="n")
        v = scanp.tile([128, CC], f16, tag="v")
        q = scanp.tile([128, CC], f16, tag="q")
        u = scanp.tile([128, CC], f16, tag="u")

        # r|z = sigmoid(gi_r + a*h | gi_z + b*h)  (args finished in PSUM)
        nc.scalar.activation(rz[:].rearrange("p (g x) -> p g x", g=2),
                             gi_rz_ap(w), AF.Sigmoid)
        # v = (h*c)*r  [+ d*r if d != 0]
        nc.vector.scalar_tensor_tensor(v[:], h, c_s, rz[:, 0:CC], AL.mult, AL.mult)
        if d_nonzero:
            nc.vector.scalar_tensor_tensor(v[:], rz[:, 0:CC], d_s, v[:],
                                           AL.mult, AL.add)
        # q = z*h (feeds early assists for w+1)
        nc.vector.tensor_tensor(q[:], rz[:, CC:], h, AL.mult)
        # gi_n[w] += c... (c folded into v) -> accumulate v
        nc.tensor.matmul(gi_ap(w, 2), diag(D_C), v[:], start=False, stop=True,
                         skip_group_check=True)
        # n = tanh(gi_n + v)
        # (early assists emitted after tanh: tile-granular dep tracking would
        # otherwise stall tanh on them)
        nc.scalar.activation(n_t[:], gi_ap(w, 2), AF.Tanh)
        if w + 1 < W:
            nc.tensor.matmul(gi_ap(w + 1, 0), diag(D_A), q[:], start=False,
                             stop=True, skip_group_check=True)
            nc.tensor.matmul(gi_ap(w + 1, 1), diag(D_B), q[:], start=False,
                             stop=True, skip_group_check=True)
        # u = (z-1)*n
        nc.vector.scalar_tensor_tensor(u[:], rz[:, CC:], 1.0, n_t[:],
                                       AL.subtract, AL.mult)
        if w + 1 < W:
            nc.tensor.matmul(gi_ap(w + 1, 0), diag(D_NA), u[:], start=False,
                             stop=True, skip_group_check=True)
            nc.tensor.matmul(gi_ap(w + 1, 1), diag(D_NB), u[:], start=False,
                             stop=True, skip_group_check=True)
        # h' = q - u
        nc.vector.tensor_tensor(out_sb[:, (w + 1) * CC : (w + 2) * CC],
                                q[:], u[:], AL.subtract)
        for (kk, g, f) in slice_for_step(w):
            emit_main_mm(kk, g, f)

    emit_out_dma(NGRP - 1)


# v is c-folded: fold c into the v stt; diag(D_C) stays ones so the PSUM
# accumulate adds v as-is.  (D_C diag values are set to 1.0 host-side.)


def _prep_core_inputs(inputs, core):
    x = inputs["inputs"]          # (W,E,B,I,F) f32
    state = inputs["state"]       # (1,E,BI,1)
    wl = inputs["weight_linear"]  # (E,16,F)
    bl = inputs["bias_linear"]    # (E,16)
    wih = inputs["weight_ih"]     # (E,3,16)
    whh = inputs["weight_hh"]     # (E,3,1)
    bih = inputs["bias_ih"]       # (E,3)
    bhh = inputs["bias_hh"]       # (E,3)

    es = slice(core * E_LOC, (core + 1) * E_LOC)
    Wc = np.einsum("egp,epf->egf", wih[es], wl[es])          # (2,3,F)
    bc = np.einsum("egp,ep->eg", wih[es], bl[es]) + bih[es]  # (2,3)
    bc = bc.copy()
    bc[:, 0] += bhh[es][:, 0]
    bc[:, 1] += bhh[es][:, 1]
    # n-gate hh bias (d) is handled in the scan when nonzero

    xr = np.asarray(x[:, es]).reshape(W, E_LOC, PP, CC, F)
    xh = np.ascontiguousarray(xr.transpose(1, 2, 4, 0, 3)).reshape(128, F, W, CC)
    xh = xh.astype(np.float16)

    pe = np.repeat(np.arange(E_LOC), PP)  # (128,) member index per partition
    dgv = np.zeros((128, NDIAG), np.float32)
    for g in range(G):
        for f in range(F):
            dgv[:, _dslot(g, f)] = Wc[pe, g, f]
        dgv[:, _dslot(g, -1)] = bc[pe, g]
    a = whh[es][pe, 0, 0]
    b = whh[es][pe, 1, 0]
    dgv[:, D_A] = a
    dgv[:, D_B] = b
    dgv[:, D_NA] = -a
    dgv[:, D_NB] = -b
    dgv[:, D_C] = 1.0
    dgm = np.zeros((128, NDIAG, 128), np.float16)
    idx = np.arange(128)
    dgm[idx, :, idx] = dgv.astype(np.float16)
    dgm = dgm.reshape(128, NDIAG * 128)

    cstv = np.zeros((128, 7 + CC), np.float32)
    cstv[:, 0] = a
    cstv[:, 1] = b
    cstv[:, 2] = whh[es][pe, 2, 0]
    cstv[:, 3] = bhh[es][pe, 2]
    cstv[:, 4] = bc[pe, 2]
    cstv[:, 5] = -a
    cstv[:, 6] = -b
    h0 = np.asarray(state[-1, es, :, 0]).reshape(E_LOC, PP, CC)
    cstv[:, 7:] = h0.reshape(128, CC)

    return {"xh": xh, "dg": dgm, "cst": cstv}


def kernel(**inputs):
    from concourse.bass_utils import run_bass_kernel_spmd

    bhh = np.asarray(inputs["bias_hh"])
    d_nonzero = bool(np.any(bhh[:, 2] != 0))

    key = ("nc", d_nonzero)
    if key not in _CACHED:
        _CACHED[key] = _build_nc(d_nonzero)
    nc = _CACHED[key]

    in_maps = [_prep_core_inputs(inputs, c) for c in range(NCORES)]
    res = run_bass_kernel_spmd(nc, in_maps, core_ids=list(range(NCORES)))

    full = np.zeros((W, E, B, I, 1), np.float32)
    for c in range(NCORES):
        o = np.asarray(res.results[c]["out"]).reshape(E_LOC, PP, W, CC)
        o = o.transpose(2, 0, 1, 3).reshape(W, E_LOC, BI)
        full[:, c * E_LOC : (c + 1) * E_LOC] = o.reshape(W, E_LOC, B, I, 1)
    return full
